# revision 1
# baseline (speedup 1.0000x reference)
"""GCN critic network on 8 TRN2 NeuronCores (Bass/Tile).

Sharding: nodes are permuted and packed into 32-node windows, bin-balanced
by in-degree (snake over sorted degrees) so each window carries ~equal edge
count; 392 windows per core, dst-owner processes each edge. Per GCN layer:
project features on PE, scale rows by dinv, AllGather the bf16 [N,32] node
table, then aggregate per core via batched indirect-DMA row gathers (64B
rows), DVE-built one-hot selection tiles, and PE messages-stationary
matmuls accumulating 128-edge tiles into per-window [32,32] PSUM segments.
Layer 2's W2 projection is pulled past the aggregation by linearity so both
layers aggregate in 32-dim space. deg derives from CSR segment counts
(int metadata prepared on host); dinv = rsqrt(deg) computed on device.
The W2/MLP tail streams feature-major 512-node chunks. Final per-graph
sums are reduced on device; the host sums the 8 per-core partials.
"""
import numpy as np
import ml_dtypes

bf16 = ml_dtypes.bfloat16

P = 128
NC = 8
M = 32                 # window node count
K_TILES = 7            # 128-edge tiles per window
N_NODES = 100000
NPAD = 100352
S = NPAD // NC         # 12544 local node slots per core
XT = S // P            # 98
WIN = S // M           # 392 windows per core
NT = WIN * K_TILES     # 2744 tiles per core per layer
ZROW = NPAD            # zero row in gather tables
TROWS = NPAD + 1
D = 128
H1 = 32
H2 = 64
GB = 56                # tiles per indirect-gather batch
NGB = NT // GB         # 49 gather batches
CHUNK = 512            # tail chunk (nodes) = 16 windows
WPC = CHUNK // M       # 16 windows per chunk
NCH = S // CHUNK       # 24.5 -> handled via ragged last chunk
CHS = list(range(0, S, CHUNK))


def _host_prep(x, ei):
    src = np.asarray(ei[0]).astype(np.int64)
    dst = np.asarray(ei[1]).astype(np.int64)
    E = src.shape[0]
    deg_in = np.bincount(dst, minlength=N_NODES)

    # balanced node -> (window, slot) assignment: snake over degree-sorted nodes
    order = np.argsort(-deg_in, kind="stable")
    allnodes = np.concatenate([order, np.arange(N_NODES, NPAD)])
    WTOT = NC * WIN
    r = np.arange(NPAD)
    rnd = r // WTOT
    pos = r % WTOT
    wg = np.where(rnd % 2 == 0, pos, WTOT - 1 - pos)
    g2w = np.empty(NPAD, np.int64)
    g2w[allnodes] = wg
    g2slot = np.empty(NPAD, np.int64)
    g2slot[allnodes] = rnd
    core_of = g2w // WIN
    nloc = (g2w % WIN) * M + g2slot
    trow = core_of * S + nloc

    wsum = np.bincount(g2w[:N_NODES], weights=deg_in.astype(np.float64), minlength=WTOT)
    assert wsum.max() <= K_TILES * P - M, f"window overflow: {wsum.max()}"

    # per-core slot streams (window-major, per node: self slot then in-edges)
    ekey = core_of[dst] * S + nloc[dst]
    eorder = np.argsort(ekey, kind="stable")
    sk = ekey[eorder]
    ssrc = src[eorder]
    cnts = np.bincount(ekey, minlength=NC * S)
    L = cnts + 1
    Lw = L.reshape(NC * WIN, M)
    startw = np.zeros((NC * WIN, M), np.int64)
    startw[:, 1:] = np.cumsum(Lw, axis=1)[:, :-1]

    src_slot = np.full((NC, NT * P), 10_000_000, np.int32)
    dst_slot = np.full((NC, NT * P), 99, np.int16)

    all_n = np.arange(NC * S)
    core_n = all_n // S
    nl = all_n % S
    q0 = (nl // M) * (K_TILES * P) + startw[core_n * WIN + nl // M, nl % M]
    src_slot[core_n, q0] = (core_n * S + nl).astype(np.int32)
    dst_slot[core_n, q0] = (nl % M).astype(np.int16)

    grp_start = np.zeros(NC * S + 1, np.int64)
    grp_start[1:] = np.cumsum(cnts)
    k_in_run = np.arange(E) - np.repeat(grp_start[:-1], cnts)
    ecore = sk // S
    enl = sk % S
    eq = (enl // M) * (K_TILES * P) + startw[ecore * WIN + enl // M, enl % M] + 1 + k_in_run
    src_slot[ecore, eq] = trow[ssrc].astype(np.int32)
    dst_slot[ecore, eq] = (enl % M).astype(np.int16)

    src_idx = np.ascontiguousarray(src_slot.reshape(NC, NT, P).transpose(0, 2, 1))
    dstrel = np.ascontiguousarray(
        src_slot_d := dst_slot.reshape(NC, NT, P).transpose(0, 2, 1)).astype(bf16)

    g_of = np.empty((NC, S), np.int64)
    g_of[core_of, nloc] = np.arange(NPAD)
    xpad = np.concatenate([np.asarray(x, np.float32),
                           np.zeros((NPAD - N_NODES, D), np.float32)], axis=0)
    xT = xpad[g_of.reshape(-1)].reshape(NC, S, D).transpose(0, 2, 1)
    xT = np.ascontiguousarray(xT).astype(bf16)

    icnt_nm = np.ascontiguousarray(
        cnts.reshape(NC, XT, P).transpose(0, 2, 1)).astype(np.int32)

    # masks in v order (= local node order)
    gid = np.where(g_of < N_NODES, g_of // 50000, -1)       # [NC, S]
    m0 = (gid == 0).astype(np.float32)
    m1 = (gid == 1).astype(np.float32)
    mask0 = np.ascontiguousarray(m0.reshape(NC, XT, P).transpose(0, 2, 1))
    mask1 = np.ascontiguousarray(m1.reshape(NC, XT, P).transpose(0, 2, 1))

    return dict(src_idx=src_idx, dstrel=dstrel, xT=xT, icnt=icnt_nm,
                mask0=mask0, mask1=mask1)


def _build_program():
    import concourse.bass as bass
    import concourse.mybir as mybir
    import concourse.tile as tile
    from concourse import bacc

    dt = mybir.dt
    AF = mybir.ActivationFunctionType
    OP = mybir.AluOpType
    nc = bacc.Bacc("TRN2", target_bir_lowering=False, debug=False, num_devices=NC)

    def din(name, shape, dtype):
        return nc.dram_tensor(name, shape, dtype, kind="ExternalInput").ap()

    xT_in = din("xT", [P, S], dt.bfloat16)
    W1_in = din("W1", [D, H1], dt.bfloat16)
    W2_in = din("W2", [H1, H2], dt.bfloat16)
    Wm1_in = din("Wm1", [H2, H2], dt.bfloat16)
    Wm2_in = din("Wm2", [H2, 1], dt.bfloat16)
    b1_in = din("b1c", [H1, 1], dt.float32)
    b2_in = din("b2c", [H2, 1], dt.float32)
    bm1_in = din("bm1", [H2, 1], dt.float32)
    bm2_in = din("bm2", [1, 1], dt.float32)
    iota_in = din("iota", [P, M], dt.bfloat16)
    ident_in = din("ident32", [M, M], dt.float32)
    icnt_in = din("icnt", [P, XT], dt.int32)
    srcidx_in = din("src_idx", [P, NT], dt.int32)
    dstrel_in = din("dstrel", [P, NT], dt.bfloat16)
    mask0_in = din("mask0", [P, XT], dt.float32)
    mask1_in = din("mask1", [P, XT], dt.float32)

    out_ext = nc.dram_tensor("partials", [2, 1], dt.float32, kind="ExternalOutput").ap()
    import os
    KDEBUG = os.environ.get("KDEBUG", "0") == "1"
    if KDEBUG:
        dbg_dinv = nc.dram_tensor("dbg_dinv", [P, XT], dt.float32, kind="ExternalOutput").ap()
        dbg_h1s = nc.dram_tensor("dbg_h1s", [P, XT * H1], dt.bfloat16, kind="ExternalOutput").ap()
        dbg_agg = nc.dram_tensor("dbg_agg", [M, CHUNK], dt.float32, kind="ExternalOutput").ap()
        dbg_z1s = nc.dram_tensor("dbg_z1s", [P, XT * H1], dt.bfloat16, kind="ExternalOutput").ap()

    l1_local = nc.dram_tensor("l1_local", [S, H1], dt.bfloat16).ap()
    l2_local = nc.dram_tensor("l2_local", [S, H1], dt.bfloat16).ap()
    t1 = nc.dram_tensor("t1", [TROWS, H1], dt.bfloat16).ap()
    t2 = nc.dram_tensor("t2", [TROWS, H1], dt.bfloat16).ap()
    dinv_dram = nc.dram_tensor("dinv_dram", [S], dt.float32).ap()
    v_dram = nc.dram_tensor("v_dram", [S], dt.float32).ap()

    AG = [list(range(NC))]

    with tile.TileContext(nc) as tc:
        with (
            tc.tile_pool(name="const", bufs=1) as cp,
            tc.tile_pool(name="big", bufs=1) as bigp,
            tc.tile_pool(name="msgs", bufs=8) as msgsp,
            tc.tile_pool(name="oh", bufs=4) as ohp,
            tc.tile_pool(name="work", bufs=2) as wp,
            tc.tile_pool(name="chunk", bufs=3) as chp,
            tc.tile_pool(name="psA", bufs=3, space="PSUM") as psA,
            tc.tile_pool(name="psU", bufs=3, space="PSUM") as psU,
        ):
            # ---------------- phase A ----------------
            def load(pool, shape, dtype, src, tag):
                t = pool.tile(shape, dtype, tag=tag)
                nc.sync.dma_start(out=t[:], in_=src)
                return t

            iota_s = load(cp, [P, M], dt.bfloat16, iota_in[:, :], "iota")
            ident_s = load(cp, [M, M], dt.float32, ident_in[:, :], "ident")
            icnt_s = load(cp, [P, XT], dt.int32, icnt_in[:, :], "icnt")
            srcidx_s = load(cp, [P, NT], dt.int32, srcidx_in[:, :], "srcidx")
            dstrel_s = load(cp, [P, NT], dt.bfloat16, dstrel_in[:, :], "dstrelc")
            xT_s = load(bigp, [P, S], dt.bfloat16, xT_in[:, :], "xT")
            W1_s = load(cp, [D, H1], dt.bfloat16, W1_in[:, :], "W1")
            W2_s = load(cp, [H1, H2], dt.bfloat16, W2_in[:, :], "W2")
            Wm1_s = load(cp, [H2, H2], dt.bfloat16, Wm1_in[:, :], "Wm1")
            Wm2_s = load(cp, [H2, 1], dt.bfloat16, Wm2_in[:, :], "Wm2")
            b1_s = load(cp, [H1, 1], dt.float32, b1_in[:, :], "b1")
            b2_s = load(cp, [H2, 1], dt.float32, b2_in[:, :], "b2")
            bm1_s = load(cp, [H2, 1], dt.float32, bm1_in[:, :], "bm1")
            bm2_s = load(cp, [1, 1], dt.float32, bm2_in[:, :], "bm2")
            m0_s = load(cp, [P, XT], dt.float32, mask0_in[:, :], "m0")
            m1_s = load(cp, [P, XT], dt.float32, mask1_in[:, :], "m1")

            zrow_s = cp.tile([1, H1], dt.bfloat16)
            nc.vector.memset(zrow_s[:], 0.0)
            nc.sync.dma_start(out=t1[NPAD:NPAD + 1, :], in_=zrow_s[:])
            nc.sync.dma_start(out=t2[NPAD:NPAD + 1, :], in_=zrow_s[:])

            # dinv = rsqrt(icnt+1), Newton-refined
            degf = wp.tile([P, XT], dt.float32, tag="deg")
            nc.vector.tensor_scalar(out=degf[:], in0=icnt_s[:], scalar1=1.0,
                                    scalar2=None, op0=OP.add)
            rec = wp.tile([P, XT], dt.float32, tag="rec")
            nc.vector.reciprocal(out=rec[:], in_=degf[:])
            y0 = wp.tile([P, XT], dt.float32, tag="y0")
            nc.scalar.activation(out=y0[:], in_=rec[:], func=AF.Sqrt)
            tmp = wp.tile([P, XT], dt.float32, tag="nt")
            nc.vector.tensor_tensor(out=tmp[:], in0=y0[:], in1=y0[:], op=OP.mult)
            nc.vector.tensor_tensor(out=tmp[:], in0=tmp[:], in1=degf[:], op=OP.mult)
            nc.vector.tensor_scalar(out=tmp[:], in0=tmp[:], scalar1=-0.5,
                                    scalar2=1.5, op0=OP.mult, op1=OP.add)
            dinv_nm = cp.tile([P, XT], dt.float32)
            nc.vector.tensor_tensor(out=dinv_nm[:], in0=y0[:], in1=tmp[:], op=OP.mult)
            nc.sync.dma_start(
                out=dinv_dram[:].rearrange("(t p) -> p t", p=P), in_=dinv_nm[:])

            # ---------------- phase B: h1s ----------------
            h1s_all = bigp.tile([P, XT * H1], dt.bfloat16, tag="h1sall")
            for t in range(XT):
                psb = psU.tile([P, H1], dt.float32, space="PSUM", tag="u")
                nc.tensor.matmul(out=psb[:], lhsT=xT_s[:, t * P:(t + 1) * P],
                                 rhs=W1_s[:], start=True, stop=True)
                nc.vector.tensor_scalar(out=h1s_all[:, t * H1:(t + 1) * H1],
                                        in0=psb[:], scalar1=dinv_nm[:, t:t + 1],
                                        scalar2=None, op0=OP.mult)
            nc.sync.dma_start(
                out=l1_local[:, :].rearrange("(t p) f -> p t f", p=P),
                in_=h1s_all[:].rearrange("p (t f) -> p t f", f=H1))
            if KDEBUG:
                nc.sync.dma_start(out=dbg_dinv[:, :], in_=dinv_nm[:])
                nc.sync.dma_start(out=dbg_h1s[:, :], in_=h1s_all[:])

            nc.gpsimd.collective_compute(
                "AllGather", OP.bypass, replica_groups=AG,
                ins=[l1_local[:, :]], outs=[t1[0:NPAD, :]])

            # dinv broadcast [M, S] f32 (partition-replicated)
            dinvb = bigp.tile([M, S], dt.float32, tag="dinvb")
            nc.sync.dma_start(
                out=dinvb[:], in_=dinv_dram[None, :].to_broadcast([M, S]))

            # ---------------- aggregation loop ----------------
            msgs_tiles = []
            for _ in range(8):
                mtile = msgsp.tile([P, H1], dt.bfloat16, tag="m")
                nc.vector.memset(mtile[:], 0.0)
                msgs_tiles.append(mtile)

            def aggregate(table_ap, consume_chunk):
                """consume_chunk(ch_idx, agg_chunk_tile, cw) called per 512-node chunk."""
                agg_ch = None
                for gb in range(NGB):
                    oh = ohp.tile([P, GB * M], dt.bfloat16, tag="oh")
                    nc.vector.tensor_tensor(
                        out=oh[:].rearrange("p (t j) -> p t j", t=GB),
                        in0=dstrel_s[:, gb * GB:(gb + 1) * GB].to_broadcast([P, GB, M]),
                        in1=iota_s[:][:, None, :].to_broadcast([P, GB, M]),
                        op=OP.is_equal)
                    for j in range(GB):
                        tg = gb * GB + j
                        w = tg // K_TILES
                        jj = tg % K_TILES
                        if jj == 0 and w % WPC == 0:
                            agg_ch = chp.tile([M, CHUNK], dt.float32, tag="aggch")
                        if jj == 0:
                            ps = psA.tile([M, M], dt.float32, space="PSUM", tag="agg")
                        mt = msgs_tiles[tg % 8]
                        nc.gpsimd.indirect_dma_start(
                            out=mt[:], out_offset=None, in_=table_ap,
                            in_offset=bass.IndirectOffsetOnAxis(
                                ap=srcidx_s[:, tg:tg + 1], axis=0),
                            bounds_check=NPAD - 1, oob_is_err=False)
                        nc.tensor.matmul(
                            out=ps[:], lhsT=mt[:],
                            rhs=oh[:, j * M:(j + 1) * M],
                            start=(jj == 0), stop=(jj == K_TILES - 1))
                        if jj == K_TILES - 1:
                            wc = w % WPC
                            nc.scalar.copy(out=agg_ch[:, wc * M:(wc + 1) * M],
                                           in_=ps[:])
                            if wc == WPC - 1 or w == WIN - 1:
                                ci = w // WPC
                                consume_chunk(ci, agg_ch, (wc + 1) * M)

            # ---------------- L1: aggregate + tail -> l2 table --------------
            z1s_nm = bigp.tile([P, XT * H1], dt.bfloat16, tag="z1snm")

            def l1_chunk(ci, agg_ch, cw):
                ch = ci * CHUNK
                if KDEBUG and ci == 0:
                    nc.sync.dma_start(out=dbg_agg[:, :], in_=agg_ch[:, :])
                tb = chp.tile([M, CHUNK], dt.float32, tag="t1a")
                nc.vector.tensor_tensor(out=tb[:, :cw], in0=agg_ch[:, :cw],
                                        in1=dinvb[:, ch:ch + cw], op=OP.mult)
                tz = chp.tile([M, CHUNK], dt.float32, tag="t1b")
                nc.scalar.activation(out=tz[:, :cw], in_=tb[:, :cw], func=AF.Relu,
                                     bias=b1_s[:, 0:1], scale=1.0)
                z1s = chp.tile([M, CHUNK], dt.float32, tag="t1c")
                nc.vector.tensor_tensor(out=z1s[:, :cw], in0=tz[:, :cw],
                                        in1=dinvb[:, ch:ch + cw], op=OP.mult)
                # transpose 128-node blocks to node-major bf16 staging
                for k in range(cw // P):
                    pst = psU.tile([P, M], dt.float32, space="PSUM", tag="u")
                    nc.tensor.transpose(out=pst[:], in_=z1s[:, k * P:(k + 1) * P],
                                        identity=ident_s[:])
                    t = ci * (CHUNK // P) + k
                    nc.scalar.copy(out=z1s_nm[:, t * H1:(t + 1) * H1], in_=pst[:])

            aggregate(t1[:, :], l1_chunk)
            nc.sync.dma_start(
                out=l2_local[:, :].rearrange("(t p) f -> p t f", p=P),
                in_=z1s_nm[:].rearrange("p (t f) -> p t f", f=H1))
            if KDEBUG:
                nc.sync.dma_start(out=dbg_z1s[:, :], in_=z1s_nm[:])

            nc.gpsimd.collective_compute(
                "AllGather", OP.bypass, replica_groups=AG,
                ins=[l2_local[:, :]], outs=[t2[0:NPAD, :]])

            # ---------------- L2: aggregate + MLP tail ----------------------
            def l2_chunk(ci, agg_ch, cw):
                ch = ci * CHUNK
                a2 = chp.tile([M, CHUNK], dt.float32, tag="t2a")
                nc.vector.tensor_tensor(out=a2[:, :cw], in0=agg_ch[:, :cw],
                                        in1=dinvb[:, ch:ch + cw], op=OP.mult)
                a2b = chp.tile([M, CHUNK], dt.bfloat16, tag="t2b")
                nc.scalar.copy(out=a2b[:, :cw], in_=a2[:, :cw])
                psz = psU.tile([H2, CHUNK], dt.float32, space="PSUM", tag="u")
                nc.tensor.matmul(out=psz[:, :cw], lhsT=W2_s[:], rhs=a2b[:, :cw],
                                 start=True, stop=True)
                z2 = chp.tile([H2, CHUNK], dt.bfloat16, tag="t2c")
                nc.scalar.activation(out=z2[:, :cw], in_=psz[:, :cw], func=AF.Relu,
                                     bias=b2_s[:, 0:1], scale=1.0)
                psm = psU.tile([H2, CHUNK], dt.float32, space="PSUM", tag="u")
                nc.tensor.matmul(out=psm[:, :cw], lhsT=Wm1_s[:], rhs=z2[:, :cw],
                                 start=True, stop=True)
                m1 = chp.tile([H2, CHUNK], dt.bfloat16, tag="t2d")
                nc.scalar.activation(out=m1[:, :cw], in_=psm[:, :cw], func=AF.Relu,
                                     bias=bm1_s[:, 0:1], scale=1.0)
                psv = psU.tile([1, CHUNK], dt.float32, space="PSUM", tag="u")
                nc.tensor.matmul(out=psv[:, :cw], lhsT=Wm2_s[:], rhs=m1[:, :cw],
                                 start=True, stop=True)
                vout = chp.tile([1, CHUNK], dt.float32, tag="t2e")
                nc.vector.tensor_scalar(out=vout[:, :cw], in0=psv[:, :cw],
                                        scalar1=bm2_s[0:1, 0:1], scalar2=None,
                                        op0=OP.add)
                nc.sync.dma_start(out=v_dram[ch:ch + cw], in_=vout[0:1, :cw])

            aggregate(t2[:, :], l2_chunk)

            # ---------------- final per-graph reduction ---------------------
            v2 = wp.tile([P, XT], dt.float32, tag="v2")
            nc.sync.dma_start(out=v2[:], in_=v_dram[:].rearrange("(t p) -> p t", p=P))
            red = wp.tile([P, 2], dt.float32, tag="red")
            vm = wp.tile([P, XT], dt.float32, tag="vm")
            nc.vector.tensor_tensor(out=vm[:], in0=v2[:], in1=m0_s[:], op=OP.mult)
            nc.vector.tensor_reduce(out=red[:, 0:1], in_=vm[:],
                                    axis=mybir.AxisListType.X, op=OP.add)
            vm2 = wp.tile([P, XT], dt.float32, tag="vm2")
            nc.vector.tensor_tensor(out=vm2[:], in0=v2[:], in1=m1_s[:], op=OP.mult)
            nc.vector.tensor_reduce(out=red[:, 1:2], in_=vm2[:],
                                    axis=mybir.AxisListType.X, op=OP.add)
            ones = wp.tile([P, 1], dt.float32, tag="ones")
            nc.vector.memset(ones[:], 1.0)
            psf = psU.tile([2, 1], dt.float32, space="PSUM", tag="u")
            nc.tensor.matmul(out=psf[:], lhsT=red[:], rhs=ones[:],
                             start=True, stop=True)
            outs = wp.tile([2, 1], dt.float32, tag="outs")
            nc.scalar.copy(out=outs[:], in_=psf[:])
            nc.sync.dma_start(out=out_ext[:, :], in_=outs[:])

    nc.compile()
    return nc


_NC_CACHE = None


def kernel(x, W1c, b1c, W2c, b2c, Wm1, bm1, Wm2, bm2, ei, num_nodes, _trace=False):
    global _NC_CACHE
    from concourse.bass_utils import run_bass_kernel_spmd

    x = np.asarray(x)
    prep = _host_prep(x, np.asarray(ei))

    W1b = np.asarray(W1c, np.float32).astype(bf16)
    W2b = np.asarray(W2c, np.float32).astype(bf16)
    Wm1b = np.asarray(Wm1, np.float32).astype(bf16)
    Wm2b = np.asarray(Wm2, np.float32).astype(bf16)
    b1v = np.asarray(b1c, np.float32).reshape(H1, 1)
    b2v = np.asarray(b2c, np.float32).reshape(H2, 1)
    bm1v = np.asarray(bm1, np.float32).reshape(H2, 1)
    bm2v = np.asarray(bm2, np.float32).reshape(1, 1)
    iota = np.ascontiguousarray(np.broadcast_to(np.arange(M), (P, M))).astype(bf16)
    ident = np.eye(M, dtype=np.float32)

    if _NC_CACHE is None:
        _NC_CACHE = _build_program()
    nc = _NC_CACHE

    in_maps = []
    for c in range(NC):
        in_maps.append({
            "xT": prep["xT"][c],
            "W1": W1b, "W2": W2b, "Wm1": Wm1b, "Wm2": Wm2b,
            "b1c": b1v, "b2c": b2v, "bm1": bm1v, "bm2": bm2v,
            "iota": iota, "ident32": ident,
            "icnt": prep["icnt"][c],
            "src_idx": prep["src_idx"][c],
            "dstrel": prep["dstrel"][c],
            "mask0": prep["mask0"][c],
            "mask1": prep["mask1"][c],
        })

    res = run_bass_kernel_spmd(nc, in_maps, core_ids=list(range(NC)),
                               trace=_trace)
    tot = np.zeros(2, np.float64)
    for c in range(NC):
        tot += res.results[c]["partials"].reshape(2).astype(np.float64)
    nn = int(np.asarray(num_nodes).reshape(-1)[0])
    out = (tot / nn).astype(np.float32)
    if _trace:
        return out, res
    return out

